# revision 51
# baseline (speedup 1.0000x reference)
"""Trainium2 Bass kernel for the diagonal complex linear recurrence (SSM scan).

Problem: out[t, d] = z_d * out[t-1, d] + x[t, d],  z_d = exp(-exp(size_d) + i*theta_d)
         x: [T=8192, D=2048] f32, out: [T, D] complex64.

Strategy (v3, fp16):
  - Shard channels D across 8 cores (256 each), pure model parallelism.
  - Per core, layout [channels(partitions), time(free)].  Per time-chunk of
    length L the complex scan splits via a local phase twist into two REAL
    first-order scans (hardware tensor_tensor_scan, 2.03 cyc/elem):
        v[jL+l] = e^{i*theta*l} * W_j[l]
        W_j[l]  = r * W_j[l-1] + e^{-i*theta*l} * x[jL+l],   r = |z|
  - fp16 everywhere for the DVE 2x packed mode; the scan decay operand is a
    stride-0 broadcast of r [128,1] fp32 (exact r^k chain).
  - Sign trick: scan the negated imag chain (u_im' = +x*sin => W_im' = -W_im):
        v_re = cos.W_re + sin.W_im'   (eye, eye)
        v_im = sin.W_re - cos.W_im'   (eye, eyeNEG)
    combines run on the PE as identity matmul accumulation into PSUM;
    ScalarE copies PSUM->SBUF (fp32->fp16).
  - W_re/W_im' live in one contiguous tile [128, 2L]; with paired tables
    (cos||sin) and (sin||cos) each untwist needs just 2 full-length TTs:
        tA = (cos||sin) . (Wre||Wim')  = t1 || t2
        tB = (sin||cos) . (Wre||Wim')  = t3 || t4
  - First chunk is split so compute starts during the table DMA; the last
    chunk is split so the scan->untwist->PE->copy->DMA tail overlaps.
"""

import os
import sys

import numpy as np

for _p in ("/opt/trn_rl_repo", "/root/.axon_site/_ro/trn_rl_repo"):
    if os.path.isdir(_p) and _p not in sys.path:
        sys.path.append(_p)

import concourse.bacc as bacc
import concourse.mybir as mybir
from concourse import bass_utils
from concourse.tile import TileContext

T = 8192
D = 2048
NCORES = 8
DS = D // NCORES          # 256 channels per core
G = DS // 128             # partition groups per core (2)
L = 2048                  # twist-chunk length (scan segment)
C = T // L                # chunks (4)
F32 = mybir.dt.float32
F16 = mybir.dt.float16

_PROGRAM = None


def _build_program():
    nc = bacc.Bacc("TRN2", target_bir_lowering=False)

    xT = nc.dram_tensor("xT", (DS, T), F16, kind="ExternalInput")
    # paired twist table: [cos(th*l) || sin(th*l)]
    cossin = nc.dram_tensor("cossin", (DS, 2 * L), F16, kind="ExternalInput")
    bnd = nc.dram_tensor("bnd", (DS, 4), F32, kind="ExternalInput")  # cL,sL,nsL,r
    eye = nc.dram_tensor("eye", (128, 256), F16, kind="ExternalInput")  # [I, -I]
    out_re = nc.dram_tensor("out_re", (DS, T), F16, kind="ExternalOutput")
    out_im = nc.dram_tensor("out_im", (DS, T), F16, kind="ExternalOutput")

    mult = mybir.AluOpType.mult
    add = mybir.AluOpType.add
    ident = mybir.ActivationFunctionType.Identity
    MMF = 512  # per-matmul free dim (one PSUM bank)

    # work pieces: (chunk, lo, hi, group) with twist-phase l = local offset in
    # chunk.  Chunk 0 is group-staggered and split once so the DVE starts
    # after only group-0's lead DMA; later chunks run full-length per group
    # (the tail drains at PE/copy/DMA block granularity, not DVE pieces).
    L0 = 512
    LT = 3 * L // 4   # tail split: last piece per group avoids the PE path
    # both groups' lead pieces run before either bulk piece, so the DVE has
    # ~9us of work in flight while the 6 MB of bulk tables streams in.
    pieces = [(0, 0, L0, 0), (0, 0, L0, 1), (0, L0, L, 0), (0, L0, L, 1)]
    for j in range(1, C - 1):
        pieces += [(j, 0, L, 0), (j, 0, L, 1)]
    pieces += [(C - 1, 0, LT, 0), (C - 1, 0, LT, 1),
               (C - 1, LT, L, 0), (C - 1, LT, L, 1)]

    with TileContext(nc) as tc:
        with tc.tile_pool(name="tabs", bufs=1) as tpool, \
             tc.tile_pool(name="xp", bufs=2) as xpool, \
             tc.tile_pool(name="work", bufs=2) as pool, \
             tc.tile_pool(name="tpool2", bufs=3) as tabpool, \
             tc.tile_pool(name="outp", bufs=3) as opool, \
             tc.tile_pool(name="kpool", bufs=4) as kpool, \
             tc.tile_pool(name="psum", bufs=2, space="PSUM") as ppool:
            # group-staggered prologue: group 0's lead DMAs first (so the DVE
            # starts after ~0.4 MB), then group 0 bulk, then group 1.
            tabs = []
            x0 = []
            # compute-critical DMAs first: each descriptor costs ~650 ns of
            # serial Sync-queue issue time, so x/table leads go ahead of the
            # tiny bnd transfer, and each cos+sin pair is one 3D-AP DMA.
            for g in range(G):
                pg = slice(g * 128, (g + 1) * 128)
                bnd_t = tpool.tile([128, 4], F32, name=f"bnd_t{g}")
                cs_t = tpool.tile([128, 2 * L], F16, name=f"cs_t{g}")
                sc_t = tpool.tile([128, 2 * L], F16, name=f"sc_t{g}")
                xt = xpool.tile([128, L], F16, name="xt", tag=f"xt{g}")
                cs3d = cs_t[:].rearrange("p (s n) -> p s n", s=2)
                src3d = cossin[pg, :].rearrange("p (s n) -> p s n", s=2)
                nc.sync.dma_start(xt[:, 0:L0], xT[pg, 0:L0])
                nc.sync.dma_start(cs3d[:, :, 0:L0], src3d[:, :, 0:L0])
                nc.sync.dma_start(bnd_t[:], bnd[pg, :])
                tabs.append([cs_t, sc_t, bnd_t[:, 3:4], bnd_t])
                x0.append(xt)
            eye_t = tpool.tile([128, 256], F16, name="eye_t")
            nc.sync.dma_start(eye_t[:], eye[:])
            for g in range(G):
                pg = slice(g * 128, (g + 1) * 128)
                cs_t, sc_t = tabs[g][0], tabs[g][1]
                cs3d = cs_t[:].rearrange("p (s n) -> p s n", s=2)
                src3d = cossin[pg, :].rearrange("p (s n) -> p s n", s=2)
                nc.sync.dma_start(x0[g][:, L0:L], xT[pg, L0:L])
                nc.sync.dma_start(cs3d[:, :, L0:L], src3d[:, :, L0:L])
                # swapped table (sin||cos) built on ScalarE once per group,
                # so tB is a single full-length TT on full chunks.
                nc.scalar.copy(sc_t[:, 0:L], cs_t[:, L:2 * L])
                nc.scalar.copy(sc_t[:, L:2 * L], cs_t[:, 0:L])
            eyeP = eye_t[:, 0:128]
            eyeN = eye_t[:, 128:256]

            K = [[None, None] for _ in range(G)]   # fp32 carries per group
            cur = [None] * G                       # (xt, wri) per group
            for (j, a, b, g) in pieces:
                    pg = slice(g * 128, (g + 1) * 128)
                    cs_t, sc_t, rb_t, bnd_t = tabs[g]
                    ts = slice(j * L + a, j * L + b)
                    n = b - a

                    if a == 0:
                        if j == 0:
                            xt = x0[g]
                        else:
                            xt = xpool.tile([128, L], F16, name="xt",
                                            tag=f"xt{g}")
                            nc.sync.dma_start(xt[:], xT[pg, j * L:(j + 1) * L])
                        # W_re || W_im' in one contiguous tile [128, 2L]
                        wri = pool.tile([128, 2 * L], F16, name="wri",
                                        tag=f"wri{g}")
                        cur[g] = (xt, wri)
                    else:
                        xt, wri = cur[g]
                    wre = wri[:, 0:L]
                    wim = wri[:, L:2 * L]

                    # twist: u_re = x*cos, u_im' = x*sin   (fp16 2x).
                    # (GPSIMD offload of these was tried and is 1.5x worse
                    # end-to-end: Pool TT is ~4x slower than DVE and steals
                    # the shared SBUF port from the DVE's scans.)
                    # Full chunks do both components in ONE TT: the x operand
                    # broadcasts over the paired (cos||sin) table via a
                    # stride-0 middle dim.
                    uri = pool.tile([128, 2 * n], F16, name="uri",
                                    tag=f"uri{g}")
                    ure = uri[:, 0:n]
                    uim = uri[:, n:2 * n]
                    if a == 0 and b == L:
                        nc.vector.tensor_tensor(
                            uri[:].rearrange("p (s n) -> p s n", s=2),
                            cs_t[:].rearrange("p (s n) -> p s n", s=2),
                            xt[:].unsqueeze(1).broadcast_to((128, 2, L)),
                            op=mult)
                    else:
                        nc.vector.tensor_tensor(ure, xt[:, a:b],
                                                cs_t[:, a:b], op=mult)
                        nc.vector.tensor_tensor(uim, xt[:, a:b],
                                                cs_t[:, L + a:L + b],
                                                op=mult)

                    # real scans with decay r (stride-0 broadcast, fp32 chain)
                    rbb = rb_t.broadcast_to((128, n))
                    if a == 0:
                        init_re = 0.0 if j == 0 else K[g][0][:]
                        init_im = 0.0 if j == 0 else K[g][1][:]
                    else:
                        init_re = wre[:, a - 1:a]
                        init_im = wim[:, a - 1:a]
                    nc.vector.tensor_tensor_scan(
                        wre[:, a:b], rbb, ure, init_re, op0=mult, op1=add)
                    nc.vector.tensor_tensor_scan(
                        wim[:, a:b], rbb, uim, init_im, op0=mult, op1=add)

                    # carry rotation for next chunk (ScalarE, fp32):
                    #   Kre = cL*WreE + sL*WimE';  Kim' = cL*WimE' - sL*WreE
                    if b == L and j < C - 1:
                        cL, sL = bnd_t[:, 0:1], bnd_t[:, 1:2]
                        nsL = bnd_t[:, 2:3]
                        wreE, wimE = wre[:, L - 1:L], wim[:, L - 1:L]
                        tmp1 = kpool.tile([128, 1], F32, name="tmp1", tag="t1")
                        tmp2 = kpool.tile([128, 1], F32, name="tmp2", tag="t2")
                        kre = kpool.tile([128, 1], F32, name="kre", tag="kre")
                        kim = kpool.tile([128, 1], F32, name="kim", tag="kim")
                        nc.scalar.activation(tmp1[:], wreE, ident, scale=cL)
                        nc.scalar.activation(kre[:], wimE, ident,
                                             scale=sL, bias=tmp1[:])
                        nc.scalar.activation(tmp2[:], wreE, ident, scale=nsL)
                        nc.scalar.activation(kim[:], wimE, ident,
                                             scale=cL, bias=tmp2[:])
                        K[g][0], K[g][1] = kre, kim

                    # untwist (fp16 2x):
                    #   tA = (cos||sin) . (Wre||Wim') = t1 || t2
                    #   tB = (sin . Wre) || (cos . Wim') = t3 || t4
                    # tA is one full-length TT when the piece spans the chunk.
                    tA = tabpool.tile([128, 2 * n], F16, name="tA", tag="tA")
                    tB = tabpool.tile([128, 2 * n], F16, name="tB", tag="tB")
                    if a == 0 and b == L:
                        nc.vector.tensor_tensor(
                            tA[:], cs_t[:], wri[:], op=mult)
                        nc.vector.tensor_tensor(
                            tB[:], sc_t[:], wri[:], op=mult)
                    else:
                        nc.vector.tensor_tensor(
                            tA[:, 0:n], cs_t[:, a:b], wre[:, a:b], op=mult)
                        nc.vector.tensor_tensor(
                            tA[:, n:2 * n], cs_t[:, L + a:L + b],
                            wim[:, a:b], op=mult)
                        nc.vector.tensor_tensor(
                            tB[:, 0:n], cs_t[:, L + a:L + b], wre[:, a:b],
                            op=mult)
                        nc.vector.tensor_tensor(
                            tB[:, n:2 * n], cs_t[:, a:b], wim[:, a:b],
                            op=mult)

                    ore = opool.tile([128, n], F16, name="ore", tag="ore")
                    oim = opool.tile([128, n], F16, name="oim", tag="oim")
                    if j == C - 1 and a > 0:
                        # tail pieces: combine on DVE and DMA straight out, so
                        # nothing drains through PE/PSUM/ScalarE after the
                        # last DVE op.
                        nc.vector.tensor_tensor(
                            ore[:], tA[:, 0:n], tA[:, n:2 * n], op=add)
                        nc.vector.tensor_tensor(
                            oim[:], tB[:, 0:n], tB[:, n:2 * n],
                            op=mybir.AluOpType.subtract)
                        nc.sync.dma_start(out_re[pg, ts], ore[:])
                        nc.sync.dma_start(out_im[pg, ts], oim[:])
                        continue
                    # combines on PE: psum_re = t1 + t2 ; psum_im = t3 - t4
                    for q in range(0, n, 1024):
                        hh = min(1024, n - q)
                        pre = ppool.tile([128, hh], F32, name="pre", tag="pre")
                        pim = ppool.tile([128, hh], F32, name="pim", tag="pim")
                        for h in range(0, hh, MMF):
                            w = min(MMF, hh - h)
                            hs = slice(q + h, q + h + w)
                            hs2 = slice(n + q + h, n + q + h + w)
                            ps = slice(h, h + w)
                            nc.tensor.matmul(pre[:, ps], eyeP, tA[:, hs],
                                             start=True, stop=False)
                            nc.tensor.matmul(pre[:, ps], eyeP, tA[:, hs2],
                                             start=False, stop=True)
                            nc.tensor.matmul(pim[:, ps], eyeP, tB[:, hs],
                                             start=True, stop=False)
                            nc.tensor.matmul(pim[:, ps], eyeN, tB[:, hs2],
                                             start=False, stop=True)
                        qs = slice(q, q + hh)
                        nc.scalar.copy(ore[:, qs], pre[:])
                        nc.scalar.copy(oim[:, qs], pim[:])
                    # one DMA per component per chunk: fewer descriptors on
                    # the serial Sync queue (PE-path chunks have ~16us of
                    # drain slack, so block-granular draining buys nothing)
                    nc.sync.dma_start(out_re[pg, ts], ore[:])
                    nc.sync.dma_start(out_im[pg, ts], oim[:])

    nc.compile()
    return nc


def _get_program():
    global _PROGRAM
    if _PROGRAM is None:
        _PROGRAM = _build_program()
    return _PROGRAM


def _host_prep(x, size, theta):
    """Per-core input maps (host-side sharding + table precompute)."""
    size64 = np.asarray(size, np.float64)
    theta64 = np.asarray(theta, np.float64)
    r64 = np.exp(-np.exp(size64))                      # [D]
    l64 = np.arange(L, dtype=np.float64)
    ang = theta64[:, None] * l64[None, :]              # [D, L]
    cosl = np.cos(ang).astype(np.float16)
    sinl = np.sin(ang).astype(np.float16)
    cossin = np.concatenate([cosl, sinl], axis=1)      # [D, 2L]
    bnd = np.zeros((D, 4), np.float32)
    bnd[:, 0] = np.cos(theta64 * L)
    bnd[:, 1] = np.sin(theta64 * L)
    bnd[:, 2] = -np.sin(theta64 * L)
    bnd[:, 3] = r64

    xh = np.asarray(x, np.float16)
    eye128 = np.eye(128, dtype=np.float16)
    eye = np.concatenate([eye128, -eye128], axis=1)    # [128, 256]
    in_maps = []
    for c in range(NCORES):
        sl = slice(c * DS, (c + 1) * DS)
        in_maps.append({
            "xT": np.ascontiguousarray(xh[:, sl].T),
            "cossin": np.ascontiguousarray(cossin[sl]),
            "bnd": np.ascontiguousarray(bnd[sl]),
            "eye": eye,
        })
    return in_maps


def _assemble(results):
    out = np.empty((T, D), np.complex64)
    for c, res in enumerate(results):
        sl = slice(c * DS, (c + 1) * DS)
        out[:, sl] = (res["out_re"].astype(np.float32)
                      + 1j * res["out_im"].astype(np.float32)).T
    return out


def run(x, size, theta, trace=False, **spmd_kwargs):
    nc = _get_program()
    in_maps = _host_prep(x, size, theta)
    res = bass_utils.run_bass_kernel_spmd(
        nc, in_maps, core_ids=list(range(NCORES)), trace=trace, **spmd_kwargs)
    return _assemble(res.results), res


def kernel(x, size, theta):
    out, _ = run(x, size, theta, trace=False)
    return out
